# revision 40
# baseline (speedup 1.0000x reference)
"""Trainium2 Bass kernel for nn_AudioModelM1 (2x Mamba2 + selu + pool + heads).

Sharding: data-parallel over batch -- 8 samples -> 8 NeuronCores, one sample
per core, no collectives.  Per-core layout is feature-major (features on SBUF
partitions, tokens on the free dim).  The selective scan uses the chunked
(quadratic-intra / recurrent-inter) Mamba2 formulation with Q=128 token
chunks; heavy math runs on the TensorEngine.

Perf notes (vs the first working version, 2.58ms -> 1.93ms):
 - silu(x) is computed as x*(1+tanh(x/2)) so every hot Activation op (exp,
   tanh, relu, square, copy) lives in ONE act-function table; the uniform
   2x/8x scale this introduces is folded into D host-side and absorbed by the
   gated RMSNorm.  softplus keeps its Ln (separate table) but only 2 Ln ops
   per block remain.
 - conv taps are 4x-mode DVE tensor_scalar ops (all-bf16, SBUF) combined by
   Pool tensor_adds; heads are processed in pairs (2p, 2p+1) stacked on the
   128 partitions so the state update / skip-term / C-scaling run at half the
   instruction count; xtok/btok transposes use the DMA xbar instead of
   PE+PSUM round trips.
 - exp(cum) row-broadcasts are built by a PE matmul against a constant
   selection matrix (selp) -- a gpsimd partition_broadcast fed by a DMA-
   staged row was racy on hardware (nondeterministic NaN).
"""
import sys
sys.path.insert(0, "/opt/trn_rl_repo")

from contextlib import ExitStack

import numpy as np
import ml_dtypes

import concourse.bass as bass
import concourse.tile as tile
from concourse import bacc, mybir
from concourse.bass_utils import run_bass_kernel_spmd

FP32 = mybir.dt.float32
BF16 = mybir.dt.bfloat16
AL = mybir.AluOpType
AF = mybir.ActivationFunctionType

D = 1024
E = 2048
NST = 64
HD = 64
H = 32
DCONV = 4
CCH = E + 2 * NST             # 2176 conv channels (17 tiles)
F = 2 * E + 2 * NST + H       # 4256 in_proj rows
L = 2048
NCORE = 8

BLK = 256
NBLK = L // BLK
Q = 128
QPB = BLK // Q

KT_D = D // 128
MT_F = 34
CT = CCH // 128
ET = E // 128
HP = 4

SELU_L = 1.0507009873554805
SELU_A = 1.6732632423543772
SELU_LA = SELU_L * SELU_A
LN_LA = float(np.log(SELU_LA))

_CACHE = {}


def _bf(x):
    return np.ascontiguousarray(np.asarray(x, np.float32).astype(ml_dtypes.bfloat16))


def _f32(x):
    return np.ascontiguousarray(np.asarray(x, np.float32))


def _prep_layer(w, suf):
    in_w = np.asarray(w["in_proj_w" + suf], np.float32)
    out_w = np.asarray(w["out_proj_w" + suf], np.float32)
    norm_w = np.asarray(w["norm_w" + suf], np.float32)
    conv_w = np.asarray(w["conv_w" + suf], np.float32)
    conv_b = np.asarray(w["conv_b" + suf], np.float32)
    dt_b = np.asarray(w["dt_bias" + suf], np.float32)
    A_log = np.asarray(w["A_log" + suf], np.float32)
    Dp = np.asarray(w["D" + suf], np.float32)

    win = in_w.T.reshape(KT_D, 128, F).transpose(1, 0, 2)
    wo = (out_w * norm_w[None, :]).T
    wout = wo.reshape(ET, 128, D).transpose(1, 0, 2)
    cw = conv_w.reshape(CT, 128, DCONV).transpose(1, 0, 2)
    cb = conv_b.reshape(CT, 128).T
    dx = np.repeat(Dp * 4.0, HD).reshape(ET, 128).T
    return {
        "win" + suf: _bf(win),
        "wout" + suf: _bf(wout),
        "cw" + suf: _f32(cw),
        "cb" + suf: _f32(cb),
        "dtb" + suf: _f32(dt_b.reshape(H, 1)),
        "A" + suf: _f32(-np.exp(A_log).reshape(H, 1)),
        "dx" + suf: _f32(dx),
    }


def _build():
    nc = bacc.Bacc("TRN2")
    dram = {}

    def din(name, shape, dt):
        dram[name] = nc.dram_tensor(name, list(shape), dt, kind="ExternalInput")
        return dram[name]

    xt = din("xt", (128, KT_D, L), BF16)
    for suf in ("1", "2"):
        din("win" + suf, (128, KT_D, F), BF16)
        din("wout" + suf, (128, ET, D), BF16)
        din("cw" + suf, (128, CT, DCONV), FP32)
        din("cb" + suf, (128, CT), FP32)
        din("dtb" + suf, (H, 1), FP32)
        din("A" + suf, (H, 1), FP32)
        din("dx" + suf, (128, ET), FP32)
    din("maskneg", (128, 128), FP32)
    din("selp", (32, 16, 128), BF16)
    din("identb", (128, 128), BF16)
    din("identf", (128, 128), FP32)
    din("onesb", (128, 1), BF16)
    din("whead", (128, KT_D, 10), FP32)
    din("bcat", (1, 10), FP32)

    u2 = nc.dram_tensor("u2spill", [128, KT_D, L], BF16)
    out_d = nc.dram_tensor("out", [1, 10], FP32, kind="ExternalOutput")

    with nc.allow_low_precision(reason="bf16 staging is intentional"), \
            tile.TileContext(nc) as tc, ExitStack() as ctx:
        pw = ctx.enter_context(tc.tile_pool(name="weights", bufs=1))
        pconst = ctx.enter_context(tc.tile_pool(name="consts", bufs=1))
        pio = ctx.enter_context(tc.tile_pool(name="io", bufs=2))
        pz = ctx.enter_context(tc.tile_pool(name="zsil", bufs=1))
        pxbc = ctx.enter_context(tc.tile_pool(name="xbcin", bufs=1))
        pxc = ctx.enter_context(tc.tile_pool(name="xconv", bufs=1))
        pg = ctx.enter_context(tc.tile_pool(name="gate", bufs=2))
        psc = ctx.enter_context(tc.tile_pool(name="scan", bufs=3))
        pxt = ctx.enter_context(tc.tile_pool(name="xtok", bufs=1))
        pcm = ctx.enter_context(tc.tile_pool(name="chunkmeta", bufs=1))
        pb2 = ctx.enter_context(tc.tile_pool(name="bcq2", bufs=2))
        psm = ctx.enter_context(tc.tile_pool(name="small", bufs=2))
        pse = ctx.enter_context(tc.tile_pool(name="selu", bufs=1))
        pstate = ctx.enter_context(tc.tile_pool(name="state", bufs=1))

        ps_mm = ctx.enter_context(tc.tile_pool(name="psmm", bufs=2, space="PSUM"))
        ps_tr = ctx.enter_context(tc.tile_pool(name="pstr", bufs=1, space="PSUM"))
        ps_g0 = ctx.enter_context(tc.tile_pool(name="psg0", bufs=1, space="PSUM"))
        ps_yp = ctx.enter_context(tc.tile_pool(name="psyp", bufs=2, space="PSUM"))
        ps_sp = ctx.enter_context(tc.tile_pool(name="pssp", bufs=1, space="PSUM"))

        w_in = pw.tile([128, KT_D, F], BF16)
        w_out = pw.tile([128, ET, D], BF16)
        cw = pw.tile([128, CT, DCONV], FP32)
        cb = pw.tile([128, CT], FP32)
        dtb = pw.tile([H, 1], FP32)
        Atile = pw.tile([H, 1], FP32)
        dxt = pw.tile([128, ET], FP32)

        maskneg = pconst.tile([128, 128], FP32)
        selp = pconst.tile([32, 16, 128], BF16)
        identf = pconst.tile([3 * H, 3 * H], FP32)
        xtail = pconst.tile([128, CT, HP], BF16)
        onesb = pconst.tile([128, 1], BF16)
        whead = pconst.tile([128, KT_D, 10], FP32)
        bcat = pconst.tile([1, 10], FP32)
        zeros32 = pconst.tile([H, Q], FP32)
        eps_t = pconst.tile([1, 1], FP32)
        ones32 = pconst.tile([H, 1], FP32)
        lnla_t = pconst.tile([128, 1], FP32)

        S = pstate.tile([128, (H // 2) * HD], BF16)  # pair-stacked: head 2p rows 0:64, 2p+1 rows 64:128, cols p*HD
        pacc = pstate.tile([128, KT_D], FP32)

        for t, name in ((maskneg, "maskneg"), (selp, "selp"),
                        (onesb, "onesb"), (whead, "whead"), (bcat, "bcat")):
            nc.sync.dma_start(t[:], dram[name][:])
        nc.sync.dma_start(identf[:], dram["identf"][0:3 * H, 0:3 * H])
        nc.vector.memset(zeros32[:], 0.0)
        nc.vector.memset(eps_t[:], 1e-5)
        nc.vector.memset(ones32[:], 1.0)
        nc.vector.memset(lnla_t[:], LN_LA)
        nc.vector.memset(pacc[:], 0.0)

        for layer in (0, 1):
            suf = "12"[layer]
            for t, name in ((w_in, "win"), (w_out, "wout"), (cw, "cw"), (cb, "cb"),
                            (dtb, "dtb"), (Atile, "A"), (dxt, "dx")):
                nc.sync.dma_start(t[:], dram[name + suf][:])
            nc.vector.memset(S[:], 0.0)

            for b in range(NBLK):
                tsl = slice(b * BLK, (b + 1) * BLK)
                u_t = pio.tile([128, KT_D, BLK], BF16)
                src = xt if layer == 0 else u2
                nc.sync.dma_start(u_t[:], src[:, :, tsl])

                # ---- in_proj
                sz = pz.tile([128, ET, BLK], BF16)          # 2*silu(z)
                xbc = pxbc.tile([128, CT, BLK + HP], BF16)  # conv input (padded)
                dtr = pcm.tile([H, BLK], FP32, tag="dtraw")
                if b == 0:
                    nc.vector.memset(xbc[:, :, 0:HP], 0.0)
                else:
                    nc.scalar.copy(xbc[:, :, 1:HP], xtail[:, :, 1:HP])
                for mt in range(MT_F):
                    mm = 128 if mt < 33 else 32
                    pmm = ps_mm.tile([mm, BLK], FP32, tag="mm")
                    for kt in range(KT_D):
                        nc.tensor.matmul(
                            pmm[:], w_in[:, kt, mt * 128:mt * 128 + mm],
                            u_t[:, kt, :], start=(kt == 0), stop=(kt == KT_D - 1))
                    if mt < ET:
                        # 2*silu(z) = z*(1+tanh(z/2)); factor absorbed by rmsnorm
                        sg = psm.tile([128, BLK], BF16, tag="ezu")
                        nc.scalar.activation(sg[:], pmm[:], AF.Tanh, scale=0.5)
                        zc = psm.tile([128, BLK], BF16, tag="zc")
                        nc.scalar.copy(zc[:], pmm[:])
                        nc.vector.scalar_tensor_tensor(
                            sz[:, mt, :], sg[:], 1.0, zc[:],
                            op0=AL.add, op1=AL.mult)
                    elif mt < 33:
                        eng = nc.vector if (mt % 2 == 0) else nc.scalar
                        if eng is nc.scalar:
                            nc.scalar.copy(xbc[:, mt - ET, HP:HP + BLK], pmm[:])
                        else:
                            nc.vector.tensor_copy(xbc[:, mt - ET, HP:HP + BLK],
                                                  pmm[:])
                    else:
                        # raw dt to sbuf; softplus deferred to the exp-table
                        # phase to avoid activation-table thrash
                        nc.vector.tensor_copy(dtr[:], pmm[:])
                nc.vector.tensor_copy(xtail[:, :, 1:HP],
                                       xbc[:, :, BLK + 1:BLK + HP])

                # ---- causal depthwise conv (+bias), then 2*silu via tanh
                # (the uniform scale is folded into D on the host side and
                # absorbed by the gated rmsnorm)
                xc = pxc.tile([128, ET, BLK], BF16)
                bc = psm.tile([128, BLK], BF16, tag="bc")
                for cp in range(9):
                    n_in = 2 if cp < 8 else 1
                    cv = pcm.tile([128, 2, BLK], BF16, tag="cv0")
                    cvb = pcm.tile([128, 2, BLK], BF16, tag="cv1")
                    cvc = pcm.tile([128, 2, BLK], BF16, tag="cv2")
                    cvd = pcm.tile([128, 2, BLK], BF16, tag="cv3")
                    for i in range(n_in):
                        ct = 2 * cp + i
                        nc.vector.tensor_scalar(cv[:, i, :],
                                                xbc[:, ct, 1:1 + BLK],
                                                cw[:, ct, 0:1], cb[:, ct:ct + 1],
                                                op0=AL.mult, op1=AL.add)
                        for k, dst in ((1, cvb), (2, cvc), (3, cvd)):
                            nc.vector.tensor_scalar(
                                dst[:, i, :], xbc[:, ct, 1 + k:1 + k + BLK],
                                cw[:, ct, k:k + 1], None, op0=AL.mult)
                    w = slice(0, n_in)
                    nc.gpsimd.tensor_add(cv[:, w, :], cv[:, w, :], cvb[:, w, :])
                    nc.gpsimd.tensor_add(cvc[:, w, :], cvc[:, w, :], cvd[:, w, :])
                    nc.gpsimd.tensor_add(cv[:, w, :], cv[:, w, :], cvc[:, w, :])
                    ec = pcm.tile([128, 2, BLK], BF16, tag="ecp")
                    if cp < 8:
                        nc.scalar.activation(ec[:, 0:2, :], cv[:, 0:2, :],
                                             AF.Tanh, scale=0.5)
                        nc.vector.scalar_tensor_tensor(
                            xc[:, 2 * cp:2 * cp + 2, :], ec[:, 0:2, :], 1.0,
                            cv[:, 0:2, :], op0=AL.add, op1=AL.mult)
                    else:
                        nc.scalar.activation(ec[:, 0, :], cv[:, 0, :], AF.Tanh,
                                             scale=0.5)
                        nc.vector.scalar_tensor_tensor(
                            bc[:], ec[:, 0, :], 1.0, cv[:, 0, :],
                            op0=AL.add, op1=AL.mult)
                ctc2 = psm.tile([128, BLK], BF16, tag="ctc")
                nc.sync.dma_start(ctc2[0:NST, :], bc[NST:128, :])
                nc.sync.dma_start(ctc2[NST:128, :], bc[NST:128, :])

                # dt = softplus(raw + dt_bias) = ln(1 + exp(raw + b)); also
                # ln(dt) for folding into the decay-exp bias
                dt_sb = pcm.tile([H, BLK], FP32, tag="dt")
                nc.scalar.activation(dtr[:], dtr[:], AF.Exp, bias=dtb[:])
                nc.scalar.activation(dt_sb[:], dtr[:], AF.Ln, bias=ones32[:])

                alog = pcm.tile([H, BLK], FP32, tag="alog")
                nc.vector.tensor_scalar(alog[:], dt_sb[:], Atile[:], None,
                                        op0=AL.mult)

                scaleb = psm.tile([128, BLK], BF16, tag="scaleb")
                ssqb = psm.tile([1, BLK], FP32, tag="ssqb")
                g_sb = pg.tile([128, ET, BLK], BF16)

                for qi in range(QPB):
                    qsl = slice(qi * Q, (qi + 1) * Q)
                    cum = pcm.tile([H, Q], FP32, tag="cum")
                    nc.vector.tensor_tensor_scan(cum[:], alog[:, qsl], zeros32[:],
                                                 0.0, op0=AL.add, op1=AL.add)
                    ecum = pcm.tile([H, Q], BF16, tag="ecum")
                    nc.scalar.activation(ecum[:], cum[:], AF.Exp)
                    dst8 = pcm.tile([H, Q], FP32, tag="dst8")
                    nc.scalar.activation(dst8[:], cum[:], AF.Exp, scale=-1.0,
                                         bias=cum[:, Q - 1:Q])
                    sbt = pcm.tile([H, Q], FP32, tag="sbt")
                    nc.gpsimd.tensor_mul(sbt[:], dst8[:], dt_sb[:, qsl])

                    stk = pcm.tile([3 * H, Q], FP32, tag="stk")
                    nc.scalar.copy(stk[0:H, :], cum[:])
                    nc.sync.dma_start(stk[H:2 * H, :], sbt[:])
                    nc.sync.dma_start(stk[2 * H:3 * H, :], dt_sb[:, qsl])
                    ptr = ps_tr.tile([Q, 3 * H], FP32, tag="tr")
                    nc.tensor.transpose(ptr[:], stk[:], identf[0:3 * H, 0:3 * H])
                    ctall = pcm.tile([Q, 3 * H], FP32, tag="ctall")
                    nc.scalar.copy(ctall[:], ptr[:])
                    negcum = pcm.tile([Q, H], FP32, tag="negcum")
                    nc.vector.tensor_scalar(negcum[:], ctall[:, 0:H], -1.0, None,
                                            op0=AL.mult)

                    btok = pcm.tile([Q, NST], BF16, tag="btok")
                    nc.sync.dma_start_transpose(btok[:], bc[0:NST, qsl])

                    g0 = ps_g0.tile([Q, Q], FP32, tag="g0")
                    nc.tensor.matmul(g0[:], bc[0:NST, qsl], ctc2[0:NST, qsl])

                    xtok = pxt.tile([Q, E], BF16, tag="xtok")
                    for ft in range(ET):
                        nc.sync.dma_start_transpose(
                            xtok[:, ft * 128:(ft + 1) * 128], xc[:, ft, qsl])

                    g0sb = pcm.tile([Q, Q], BF16, tag="g0sb")
                    nc.scalar.copy(g0sb[:], g0[:])
                    for hg in range(H // 4):
                        h0 = hg * 4
                        stg = psc.tile([1, 4 * Q], FP32, tag="stg")
                        nc.sync.dma_start(stg[:], cum[h0:h0 + 4, :])
                        bcq4 = pb2.tile([Q, 4 * Q], FP32, tag="bcq")
                        nc.gpsimd.partition_broadcast(bcq4[:], stg[:])
                        for p2 in range(2):
                            p = hg * 2 + p2
                            psl = slice(p * HD, (p + 1) * HD)
                            bcep = ps_g0.tile([Q, Q], FP32, tag="g0")
                            nc.tensor.matmul(bcep[:], selp[:, p, :], ecum[:])
                            cpos2 = psc.tile([128, Q], BF16, tag="cpos")
                            nc.vector.tensor_mul(cpos2[:], ctc2[:, qsl], bcep[:])
                            yp2 = ps_yp.tile([128, Q], FP32, tag="yp")
                            sp2 = ps_sp.tile([128, HD], FP32, tag="sp")
                            for a in range(2):
                                h = 2 * p + a
                                ro = a * 64
                                k = 2 * p2 + a
                                csl = slice(h * HD, (h + 1) * HD)
                                ksl = slice(k * Q, (k + 1) * Q)
                                nc.gpsimd.tensor_add(bcq4[:, ksl], bcq4[:, ksl],
                                                     maskneg[:])
                                lt = psc.tile([Q, Q], BF16, tag="lt")
                                nc.scalar.activation(
                                    lt[:], bcq4[:, ksl], AF.Exp,
                                    bias=negcum[:, h:h + 1])
                                mt_t = psc.tile([Q, Q], BF16, tag="mt")
                                nc.vector.scalar_tensor_tensor(
                                    mt_t[:], g0sb[:],
                                    ctall[:, 2 * H + h:2 * H + h + 1],
                                    lt[:], op0=AL.mult, op1=AL.mult)
                                bh = psc.tile([Q, NST], BF16, tag="bh")
                                nc.vector.tensor_scalar(
                                    bh[:], btok[:], ctall[:, H + h:H + h + 1],
                                    None, op0=AL.mult)
                                nc.tensor.matmul(yp2[ro:ro + 64, :],
                                                 xtok[:, csl], mt_t[:],
                                                 start=True, stop=False)
                                nc.tensor.matmul(yp2[ro:ro + 64, :],
                                                 S[ro:ro + 64, psl],
                                                 cpos2[ro:ro + 64, :],
                                                 start=False, stop=True)
                                nc.tensor.matmul(sp2[ro:ro + 64, :], bh[:],
                                                 xtok[:, csl])
                            nc.vector.scalar_tensor_tensor(
                                S[:, psl], S[:, psl], bcep[:, Q - 1:Q], sp2[:],
                                op0=AL.mult, op1=AL.add)
                            nc.vector.scalar_tensor_tensor(
                                g_sb[:, p, qsl], xc[:, p, qsl],
                                dxt[:, p:p + 1], yp2[:],
                                op0=AL.mult, op1=AL.add)

                    # ---- gating + sum of squares (whole chunk at once)
                    g2 = pcm.tile([128, ET, Q], BF16, tag="g2q")
                    nc.vector.tensor_mul(g_sb[:, :, qsl], g_sb[:, :, qsl],
                                         sz[:, :, qsl])
                    nc.vector.tensor_mul(g2[:, :, :], g_sb[:, :, qsl],
                                         g_sb[:, :, qsl])
                    ssq = ps_sp.tile([1, Q], FP32, tag="ssq")
                    for ft in range(ET):
                        nc.tensor.matmul(ssq[:], onesb[:], g2[:, ft, :],
                                         start=(ft == 0), stop=(ft == ET - 1))
                    nc.vector.tensor_copy(ssqb[0:1, qsl], ssq[:])

                # rsqrt(mean + eps) = exp(-0.5 * ln(ssq/E + eps)), whole block
                nc.scalar.activation(ssqb[:], ssqb[:], AF.Ln,
                                     scale=1.0 / E, bias=eps_t[:])
                rs = psm.tile([1, BLK], BF16, tag="rs")
                nc.scalar.activation(rs[:], ssqb[:], AF.Exp, scale=-0.5)
                nc.gpsimd.partition_broadcast(scaleb[:], rs[:])

                # ---- out_proj + rmsnorm scale + selu (batched per block)
                t1 = pse.tile([128, ET // 2, BLK], BF16, tag="t1b")
                for mt in range(ET // 2):
                    ho = ps_mm.tile([128, BLK], FP32, tag="mm")
                    for kt in range(ET):
                        nc.tensor.matmul(ho[:], w_out[:, kt, mt * 128:(mt + 1) * 128],
                                         g_sb[:, kt, :],
                                         start=(kt == 0), stop=(kt == ET - 1))
                    hosb = psm.tile([128, BLK], BF16, tag="zc")
                    nc.scalar.copy(hosb[:], ho[:])
                    nc.gpsimd.tensor_mul(t1[:, mt, :], hosb[:], scaleb[:])
                rl = pse.tile([128, ET // 2, BLK], BF16, tag="rlb")
                nc.scalar.activation(rl[:, :, :], t1[:, :, :], AF.Relu,
                                     scale=SELU_L)
                ex = pse.tile([128, ET // 2, BLK], BF16, tag="exb")
                nc.scalar.activation(ex[:, :, :], t1[:, :, :], AF.Exp,
                                     bias=lnla_t[:])
                nc.vector.tensor_scalar_min(ex[:, :, :], ex[:, :, :], SELU_LA)
                if layer == 0:
                    nc.vector.scalar_tensor_tensor(
                        ex[:, :, :], ex[:, :, :], -SELU_LA, rl[:, :, :],
                        op0=AL.add, op1=AL.add)
                    nc.sync.dma_start(u2[:, :, tsl], ex[:, :, :])
                else:
                    for mt in range(ET // 2):
                        red = psm.tile([128, 1], FP32, tag="red")
                        nc.vector.scalar_tensor_tensor(
                            ex[:, mt, :], ex[:, mt, :], -SELU_LA, rl[:, mt, :],
                            op0=AL.add, op1=AL.add, accum_out=red[:])
                        nc.gpsimd.tensor_add(pacc[:, mt:mt + 1],
                                             pacc[:, mt:mt + 1], red[:])

        pooled = psm.tile([128, KT_D], FP32, tag="pooled")
        nc.vector.tensor_scalar(pooled[:], pacc[:], 1.0 / L, None, op0=AL.mult)
        ph = ps_sp.tile([1, 10], FP32, tag="sp")
        for kt in range(KT_D):
            nc.tensor.matmul(ph[:], pooled[:, kt:kt + 1], whead[:, kt, :],
                             start=(kt == 0), stop=(kt == KT_D - 1))
        ot = psm.tile([1, 10], FP32, tag="ot")
        nc.vector.tensor_add(ot[:], ph[:], bcat[:])
        nc.sync.dma_start(out_d[:], ot[:])

    nc.compile()
    return nc


def _host_inputs(inputs):
    m = {}
    m.update(_prep_layer(inputs, "1"))
    m.update(_prep_layer(inputs, "2"))
    j = np.arange(128)
    m["maskneg"] = _f32(np.where(j[None, :] >= j[:, None], 0.0, -1e30))
    sel = np.zeros((32, 16, 128), np.float32)
    for p in range(16):
        sel[2 * p, p, 0:64] = 1.0
        sel[2 * p + 1, p, 64:128] = 1.0
    m["selp"] = _bf(sel)
    m["identb"] = _bf(np.eye(128))
    m["identf"] = _f32(np.eye(128))
    m["onesb"] = _bf(np.ones((128, 1)))
    wcat = np.concatenate([np.asarray(inputs["w_emo"], np.float32),
                           np.asarray(inputs["w_sen"], np.float32)], 0)
    m["whead"] = _f32(wcat.T.reshape(KT_D, 128, 10).transpose(1, 0, 2))
    m["bcat"] = _f32(np.concatenate([inputs["b_emo"], inputs["b_sen"]])
                     .reshape(1, 10))
    return m


def kernel(**inputs) -> np.ndarray:
    if "nc" not in _CACHE:
        _CACHE["nc"] = _build()
    nc = _CACHE["nc"]

    x = np.asarray(inputs["x"], np.float32)
    shared = _host_inputs(inputs)
    in_maps = []
    for s in range(NCORE):
        m = dict(shared)
        xts = x[s].T.reshape(KT_D, 128, L).transpose(1, 0, 2)
        m["xt"] = _bf(xts)
        in_maps.append(m)

    res = run_bass_kernel_spmd(nc, in_maps, core_ids=list(range(NCORE)))
    out = np.concatenate([r["out"] for r in res.results], 0)
    return out.astype(np.float32)

